# revision 34
# baseline (speedup 1.0000x reference)
"""RGCN (2-layer, per-(dst,rel) mean aggregation) + triplet projection,
distributed over 8 Trainium2 NeuronCores (one SPMD Bass/Tile program).

Sharding: destination-node ranges (6250 nodes/core). Aggregate-first:
  y[dst,rel] = (1/cnt) * sum_{src} x[src]   built as one-hot "slab" matmuls
  accumulated in PSUM, then agg = sum_r y_r @ W_r + x @ root + b, ReLU,
  AllGather h. Triplet: u = h@Wp[:256]+bp, v = h@Wp[256:] per node,
  AllGather u & v, then out[e] = u[src_e] + v[dst_e] via dma_gather + add.

Gathers use gpsimd dma_gather (1024 rows per instruction, int16 indices).
The int16 limit (32767 < 50000 rows) is handled by splitting edges into lo
(src<32768, table base row 0) and hi (src>=32768, base row 17232) streams.

Edge slots are packed DENSELY: per-(w,r,reg) run sizes are the max edge
count over the 8 cores (no 128-rounding); runs share 128-row chunks. Each
(chunk, run) overlap is one matmul instance whose one-hot slab
(slab[e, dst%128] = norm, zero outside the run via a dl=200 sentinel) is
built on-chip with a single fused tensor_scalar (iota==dl)*norm op.
The instruction stream is identical on all cores; per-core variation lives
in the gather-index / dl / norm input tensors.
"""

import numpy as np
import ml_dtypes

BF16 = ml_dtypes.bfloat16

N, R, F, E, NCORES = 50000, 8, 256, 400000, 8
NC = N // NCORES             # 6250
W = (NC + 127) // 128        # 49 windows/core
NPAD = W * 128               # 6272
SPLIT = 32768
HIBASE = 17232               # hi idx = src - HIBASE (<= 32767)
PC = 8                       # gather piece = 8 chunks = 1024 rows
LAST_EXEC_NS = None
LAST_TRACE = None


def _wrap_idx(a):
    """[slots or n, 128] -> [128, n]: column j holds row j."""
    return np.ascontiguousarray(a.T)


def _wrap16(idx):
    """int16 idx [slots] -> [128, slots//16]: element i at [i%16, i//16],
    replicated across the 8 gpsimd core partition groups."""
    s = len(idx) // 16
    a = np.ascontiguousarray(idx.reshape(s, 16).T)
    return np.tile(a, (8, 1))


def _plan_agg(src, dst, et, norm):
    """Dense instance plan.  Returns:
    nchunk[2]      : chunk-grid length per region
    insts          : list of (w, r, reg, chunk, lo_in_chunk, hi_in_chunk,
                      first, last)  (common across cores)
    idx16 per core per region, dl/nm [128, n_inst] f32 per core
    """
    core = dst // NC
    percore = []
    counts = np.zeros((NCORES, W, R, 2), dtype=np.int64)
    for c in range(NCORES):
        m = np.where(core == c)[0]
        dl = dst[m] - c * NC
        w = dl >> 7
        reg = (src[m] >= SPLIT).astype(np.int64)
        percore.append((m, dl, w, reg))
        key = (w * R + et[m]) * 2 + reg
        counts[c] = np.bincount(key, minlength=W * R * 2).reshape(W, R, 2)
    S = counts.max(axis=0)                       # [W,R,2] common run sizes
    empty = S.sum(axis=2) == 0
    S[:, :, 0][empty] = 1                        # ensure >=1 instance per (w,r)

    # run offsets within each region's dense slot stream
    off = np.zeros((W, R, 2), dtype=np.int64)
    tot = [0, 0]
    for reg in range(2):
        acc = 0
        for w in range(W):
            for r in range(R):
                off[w, r, reg] = acc
                acc += S[w, r, reg]
        tot[reg] = acc
    nchunk = [(-(-tot[reg] // 128)) for reg in range(2)]

    # instance list: per (w, r): reg 0 then reg 1, chunks ascending
    insts = []
    for w in range(W):
        for r in range(R):
            items = []
            for reg in range(2):
                sz = int(S[w, r, reg])
                if sz == 0:
                    continue
                a, bnd = int(off[w, r, reg]), int(off[w, r, reg]) + sz
                for ch in range(a // 128, (bnd - 1) // 128 + 1):
                    lo = max(a, ch * 128) - ch * 128
                    hi = min(bnd, (ch + 1) * 128) - ch * 128
                    items.append([w, r, reg, ch, lo, hi, False, False])
            items[0][6] = True
            items[-1][7] = True
            insts.extend(items)
    n_inst = len(insts)

    # (w, r, reg, chunk) -> instance id, as a dense array
    nch = max(nchunk)
    lut = np.full((W * R * 2 * nch,), -1, dtype=np.int64)
    for j, (w_, r_, reg_, ch, lo, hi, _, _) in enumerate(insts):
        lut[((w_ * R + r_) * 2 + reg_) * nch + ch] = j

    idx_s, dlnm_s = [], []
    for c in range(NCORES):
        m, dl, w, regs = percore[c]
        r = et[m]
        ipair = []
        dlv = np.full((n_inst, 128), 200.0, dtype=np.float32)
        nmv = np.zeros((n_inst, 128), dtype=np.float32)
        for reg in range(2):
            slots = nchunk[reg] * 128
            idx = np.zeros(slots, dtype=np.int32)
            sel = regs == reg
            mm = m[sel]
            order = np.lexsort((dl[sel], r[sel] + R * w[sel]))
            mm = mm[order]
            wsel, rsel, dsel = w[sel][order], r[sel][order], dl[sel][order]
            runkey = wsel * R + rsel
            runstart = off[wsel, rsel, reg]
            o = np.arange(len(mm))
            starts = np.zeros(len(mm), dtype=np.int64)
            b = np.flatnonzero(np.diff(runkey)) + 1
            starts[b] = o[b]
            starts = np.maximum.accumulate(starts)
            pos = runstart + (o - starts)
            idx[pos] = src[mm] - reg * HIBASE
            ipair.append(_wrap16(idx.astype(np.int16)))
            j = lut[((wsel * R + rsel) * 2 + reg) * nch + (pos >> 7)]
            assert (j >= 0).all()
            dlv[j, pos & 127] = dsel & 127
            nmv[j, pos & 127] = norm[mm]
        idx_s.append(ipair)
        dlnm_s.append((_wrap_idx(dlv.astype(BF16)), _wrap_idx(nmv.astype(BF16))))
    return nchunk, insts, idx_s, dlnm_s


WG = (N + 127) // 128        # 391 global node windows


def _plan_stream(key):
    """Plan one triplet select-mm stream over the per-core edge shards,
    sorted by the global node-window of `key` (src or dst).

    Returns nch (chunk count), chunk_insts (per chunk: list of
    (w, first, last)), n_inst, and per-core (kq, orig):
      kq   [n_quad, 512] bf16 : instance keys (key % 128, sentinel 200),
                                4 instances per quad row
      orig [nch*128] int64    : slot -> original edge id (-1 = pad)
    """
    EC = E // NCORES
    counts = np.zeros((NCORES, WG), dtype=np.int64)
    percore = []
    for c in range(NCORES):
        ids = np.arange(c * EC, (c + 1) * EC)
        kw = key[ids] >> 7
        order = np.argsort(kw, kind="stable")
        ids = ids[order]
        percore.append(ids)
        counts[c] = np.bincount(kw[order], minlength=WG)
    S = counts.max(axis=0)
    off = np.concatenate([[0], np.cumsum(S)]).astype(np.int64)
    tot = int(off[-1])
    nch = -(-tot // 128)

    insts = []          # (w, ch, first_of_chunk, last_of_chunk)
    chunk_insts = [[] for _ in range(nch)]
    for w in range(WG):
        if S[w] == 0:
            continue
        a, bnd = int(off[w]), int(off[w + 1])
        for ch in range(a // 128, (bnd - 1) // 128 + 1):
            chunk_insts[ch].append(w)
    n_inst = 0
    inst_of = {}
    out = []
    for ch in range(nch):
        ws = chunk_insts[ch]
        if not ws:
            out.append([])
            continue
        lst = []
        for k, w in enumerate(ws):
            inst_of[(w, ch)] = n_inst
            lst.append((w, n_inst, k == 0, k == len(ws) - 1))
            n_inst += 1
        out.append(lst)
    n_quad = -(-n_inst // 4)

    plans = []
    for c in range(NCORES):
        ids = percore[c]
        kw = key[ids] >> 7
        # dense position within the common offsets
        o = np.arange(len(ids))
        starts = np.zeros(len(ids), dtype=np.int64)
        b = np.flatnonzero(np.diff(kw)) + 1
        starts[b] = o[b]
        starts = np.maximum.accumulate(starts)
        pos = off[kw] + (o - starts)
        orig = np.full(nch * 128, -1, dtype=np.int64)
        orig[pos] = ids
        kv = np.full((n_inst, 128), 200.0, dtype=np.float32)
        j = np.array([inst_of[(int(w_), int(p_) >> 7)]
                      for w_, p_ in zip(kw, pos)], dtype=np.int64)
        # key offset within its window tile; the last window's tile is
        # loaded from rows [N-128, N) so shift its keys accordingly
        kv[j, pos & 127] = key[ids] - np.minimum(kw * 128, N - 128)
        kq = np.zeros((n_quad * 4, 128), dtype=np.float32)
        kq[:n_inst] = kv
        kq = kq.reshape(n_quad, 512).astype(BF16)
        plans.append((kq, orig))
    return nch, out, n_inst, plans


def _build(nchunk, insts, su, sv):
    import concourse.bass as bass
    import concourse.bacc as bacc
    import concourse.mybir as mybir
    import concourse.tile as tile

    dt = mybir.dt
    nc = bacc.Bacc("TRN2", target_bir_lowering=False, debug=False,
                   num_devices=NCORES)
    AF = mybir.ActivationFunctionType
    n_inst = len(insts)
    nch_u, cinst_u, ninst_u = su
    nch_v, cinst_v, ninst_v = sv
    mq_u = (((ninst_u + 3) // 4) + 127) // 128
    mq_v = (((ninst_v + 3) // 4) + 127) // 128
    NPADG = WG * 128

    x16 = nc.dram_tensor("x16", [N, F], dt.bfloat16, kind="ExternalInput")
    xsh = nc.dram_tensor("xsh", [NPAD, F], dt.bfloat16, kind="ExternalInput")
    w1d = nc.dram_tensor("w1", [R, F, F], dt.bfloat16, kind="ExternalInput")
    w2d = nc.dram_tensor("w2", [R, F, F], dt.bfloat16, kind="ExternalInput")
    r1d = nc.dram_tensor("r1", [F, F], dt.bfloat16, kind="ExternalInput")
    r2d = nc.dram_tensor("r2", [F, F], dt.bfloat16, kind="ExternalInput")
    b1d = nc.dram_tensor("b1", [128, F], dt.float32, kind="ExternalInput")
    b2d = nc.dram_tensor("b2", [128, F], dt.float32, kind="ExternalInput")
    wpud = nc.dram_tensor("wpu", [F, F], dt.bfloat16, kind="ExternalInput")
    wpvd = nc.dram_tensor("wpv", [F, F], dt.bfloat16, kind="ExternalInput")
    bpd = nc.dram_tensor("bp", [128, F], dt.float32, kind="ExternalInput")
    ilo_d = nc.dram_tensor("idx_lo", [128, nchunk[0] * 8], dt.int16, kind="ExternalInput")
    ihi_d = nc.dram_tensor("idx_hi", [128, nchunk[1] * 8], dt.int16, kind="ExternalInput")
    dl_d = nc.dram_tensor("dl", [128, n_inst], dt.bfloat16, kind="ExternalInput")
    nm_d = nc.dram_tensor("nm", [128, n_inst], dt.bfloat16, kind="ExternalInput")
    kqu_d = nc.dram_tensor("kqu", [(ninst_u + 3) // 4, 512], dt.bfloat16, kind="ExternalInput")
    kqv_d = nc.dram_tensor("kqv", [(ninst_v + 3) // 4, 512], dt.bfloat16, kind="ExternalInput")
    tu_d = nc.dram_tensor("tu", [nch_u * 128, F], dt.bfloat16, kind="ExternalOutput")
    tv_d = nc.dram_tensor("tv", [nch_v * 128, F], dt.bfloat16, kind="ExternalOutput")

    rg = [list(range(NCORES))]

    with tile.TileContext(nc) as tc:
        with (
            tc.tile_pool(name="const", bufs=1) as cp,
            tc.tile_pool(name="msg", bufs=3) as msgp,
            tc.tile_pool(name="slab", bufs=4) as slabp,
            tc.tile_pool(name="yw", bufs=2) as yp,
            tc.tile_pool(name="small", bufs=4) as sp,
            tc.tile_pool(name="ps", bufs=1, space="PSUM") as psp,
            tc.tile_pool(name="psagg", bufs=1, space="PSUM") as psaggp,
            tc.tile_pool(name="psout", bufs=2, space="PSUM") as psoutp,
            tc.tile_pool(name="dram", bufs=1, space="DRAM") as dram,
        ):
            w_sb = [cp.tile([128, 16, F], dt.bfloat16, tag=f"w{i}", name=f"w{i}") for i in range(2)]
            nc.sync.dma_start(w_sb[0][:], w1d.ap().rearrange("r (h p) o -> p (r h) o", p=128))
            nc.sync.dma_start(w_sb[1][:], w2d.ap().rearrange("r (h p) o -> p (r h) o", p=128))
            rt_sb = [cp.tile([128, 2, F], dt.bfloat16, tag=f"rt{i}", name=f"rt{i}") for i in range(2)]
            nc.sync.dma_start(rt_sb[0][:], r1d.ap().rearrange("(h p) o -> p h o", p=128))
            nc.sync.dma_start(rt_sb[1][:], r2d.ap().rearrange("(h p) o -> p h o", p=128))
            b_sb = [cp.tile([128, F], dt.float32, tag=f"b{i}", name=f"b{i}") for i in range(2)]
            nc.sync.dma_start(b_sb[0][:], b1d[:])
            nc.sync.dma_start(b_sb[1][:], b2d[:])
            wpu_sb = cp.tile([128, 2, F], dt.bfloat16, tag="wpu", name="wpu")
            wpv_sb = cp.tile([128, 2, F], dt.bfloat16, tag="wpv", name="wpv")
            nc.sync.dma_start(wpu_sb[:], wpud.ap().rearrange("(h p) o -> p h o", p=128))
            nc.sync.dma_start(wpv_sb[:], wpvd.ap().rearrange("(h p) o -> p h o", p=128))
            bp_sb = cp.tile([128, F], dt.float32, tag="bp", name="bp")
            nc.sync.dma_start(bp_sb[:], bpd[:])
            ilo_sb = cp.tile([128, nchunk[0] * 8], dt.int16, tag="ilo", name="ilo")
            ihi_sb = cp.tile([128, nchunk[1] * 8], dt.int16, tag="ihi", name="ihi")
            nc.sync.dma_start(ilo_sb[:], ilo_d[:])
            nc.sync.dma_start(ihi_sb[:], ihi_d[:])
            dl_sb = cp.tile([128, n_inst], dt.bfloat16, tag="dl", name="dl")
            nm_sb = cp.tile([128, n_inst], dt.bfloat16, tag="nm", name="nm")
            nc.sync.dma_start(dl_sb[:], dl_d[:])
            nc.sync.dma_start(nm_sb[:], nm_d[:])
            iota_sb = cp.tile([128, 128], dt.bfloat16, tag="iota", name="iota")
            nc.gpsimd.iota(iota_sb[:], pattern=[[1, 128]], channel_multiplier=0,
                           allow_small_or_imprecise_dtypes=True)
            iotai = cp.tile([128, 1], dt.int32, tag="ioi", name="ioi")
            nc.gpsimd.iota(iotai[:], pattern=[[0, 1]], channel_multiplier=1)
            iotaf = cp.tile([128, 1], dt.float32, tag="iof", name="iof")
            nc.vector.tensor_copy(iotaf[:], iotai[:])
            zt_sb = cp.tile([128, F], dt.bfloat16, tag="zt", name="zt")
            nc.vector.memset(zt_sb[:], 0.0)

            h1b = dram.tile([NPAD, F], dt.bfloat16, tag="h1b", name="h1b")
            h2b = dram.tile([NPAD, F], dt.bfloat16, tag="h2b", name="h2b")
            h1f = dram.tile([N, F], dt.bfloat16, addr_space="Shared", tag="h1f", name="h1f")
            uvb = [dram.tile([NPAD, F], dt.bfloat16, tag=f"uvb{i}", name=f"uvb{i}") for i in range(2)]
            uvf = [dram.tile([N, F], dt.bfloat16, addr_space="Shared", tag=f"uvf{i}", name=f"uvf{i}")
                   for i in range(2)]

            def gather_piece(table, idx_sb, c0, nblk, tag):
                """Gather chunks [c0, c0+nblk) of a stream into an SBUF tile
                with one dma_gather (1024 rows max)."""
                t = msgp.tile([128, nblk, F], dt.bfloat16, tag=tag, name=tag)
                ni = nblk * 128
                nc.gpsimd.dma_gather(
                    t[:], table, idx_sb[:, c0 * 8:(c0 + nblk) * 8],
                    ni, ni, F)
                return t

            GS = 8   # slab instances built per DVE op pair

            def layer(li, tables, rootsrc, hout):
                idx_sb = (ilo_sb, ihi_sb)
                pieces = [{}, {}]   # region -> piece idx -> msg_tile
                sgroups = {}        # group idx -> slab tile [128, GS, 128]

                def get_piece(reg, p):
                    if p not in pieces[reg]:
                        nblk = min(PC, nchunk[reg] - p * PC)
                        mt = gather_piece(tables[reg], idx_sb[reg], p * PC,
                                          nblk, f"m{reg}")
                        pieces[reg] = {p: mt}  # keep only latest
                    return pieces[reg][p]

                def get_slab(j):
                    g = j // GS
                    if g not in sgroups:
                        ng = min(GS, len(insts) - g * GS)
                        st = slabp.tile([128, ng, 128], dt.bfloat16,
                                        tag="st", name="st")
                        nc.vector.tensor_tensor(
                            st[:],
                            iota_sb[:, None, :].broadcast_to([128, ng, 128]),
                            dl_sb[:, g * GS:g * GS + ng, None]
                            .broadcast_to([128, ng, 128]),
                            op=mybir.AluOpType.is_equal)
                        nc.vector.tensor_tensor(
                            st[:], st[:],
                            nm_sb[:, g * GS:g * GS + ng, None]
                            .broadcast_to([128, ng, 128]),
                            op=mybir.AluOpType.mult)
                        sgroups.clear()
                        sgroups[g] = st
                    return sgroups[g]

                ii = 0
                for w in range(W):
                    ps = [[psp.tile([128, 512], dt.float32, tag=f"ps{fh}{q}", name=f"ps{fh}{q}")
                           for q in range(2)] for fh in range(2)]
                    while ii < len(insts) and insts[ii][0] == w:
                        _, r, reg, ch, lo, hi, first, last = insts[ii]
                        p, b = divmod(ch, PC)
                        mt = get_piece(reg, p)
                        st = get_slab(ii)
                        for fh in range(2):
                            nc.tensor.matmul(
                                ps[fh][r // 4][:, (r % 4) * 128:(r % 4) * 128 + 128],
                                lhsT=mt[:, b, fh * 128:(fh + 1) * 128],
                                rhs=st[:, ii - (ii // GS) * GS, :],
                                start=first, stop=last)
                        ii += 1
                    yw = yp.tile([128, 2048], dt.bfloat16, tag="yw", name="yw")
                    for fh in range(2):
                        for q in range(2):
                            eng = nc.vector if q == 0 else nc.scalar
                            (eng.tensor_copy if q == 0 else eng.copy)(
                                yw[:, (fh * 2 + q) * 512:(fh * 2 + q + 1) * 512],
                                ps[fh][q][:])
                    xt = sp.tile([128, 2, 128], dt.bfloat16, tag="xt", name="xt")
                    for fh in range(2):
                        nc.sync.dma_start(
                            xt[:, fh, :],
                            rootsrc[w * 128:(w + 1) * 128, fh * 128:(fh + 1) * 128],
                            transpose=True)
                    agg = psaggp.tile([128, F], dt.float32, tag="agg", name="agg")
                    for r in range(R):
                        for fh in range(2):
                            nc.tensor.matmul(
                                agg[:], lhsT=yw[:, (fh * 8 + r) * 128:(fh * 8 + r + 1) * 128],
                                rhs=w_sb[li][:, r * 2 + fh, :],
                                start=(r == 0 and fh == 0), stop=False)
                    for fh in range(2):
                        nc.tensor.matmul(agg[:], lhsT=xt[:, fh, :],
                                         rhs=rt_sb[li][:, fh, :],
                                         start=False, stop=(fh == 1))
                    hf = sp.tile([128, F], dt.float32, tag="hf", name="hf")
                    nc.vector.tensor_tensor(hf[:], agg[:],
                                            b_sb[li][:],
                                            op=mybir.AluOpType.add)
                    hw = sp.tile([128, F], dt.bfloat16, tag="hw", name="hw")
                    nc.scalar.activation(hw[:], hf[:], AF.Relu)
                    nc.sync.dma_start(hout[w * 128:(w + 1) * 128, :], hw[:])

            layer(0, (x16.ap(), x16.ap()[HIBASE:, :]), xsh.ap(), h1b)
            nc.gpsimd.collective_compute(
                "AllGather", mybir.AluOpType.bypass, replica_groups=rg,
                ins=[h1b[0:NC, :].opt()], outs=[h1f[:].opt()])
            layer(1, (h1f[:], h1f[HIBASE:, :]), h1b[:], h2b)

            # triplet projections u, v per node tile
            for w in range(W):
                ht = sp.tile([128, 2, 128], dt.bfloat16, tag="ht", name="ht")
                for fh in range(2):
                    nc.sync.dma_start(
                        ht[:, fh, :],
                        h2b[w * 128:(w + 1) * 128, fh * 128:(fh + 1) * 128],
                        transpose=True)
                psu = psaggp.tile([128, F], dt.float32, tag="agg", name="psu")
                psv = psaggp.tile([128, F], dt.float32, tag="psv", name="psv")
                for fh in range(2):
                    nc.tensor.matmul(psu[:], lhsT=ht[:, fh, :], rhs=wpu_sb[:, fh, :],
                                     start=(fh == 0), stop=(fh == 1))
                    nc.tensor.matmul(psv[:], lhsT=ht[:, fh, :], rhs=wpv_sb[:, fh, :],
                                     start=(fh == 0), stop=(fh == 1))
                uo = sp.tile([128, F], dt.bfloat16, tag="uo", name="uo")
                nc.vector.tensor_tensor(uo[:], psu[:],
                                        bp_sb[:],
                                        op=mybir.AluOpType.add)
                vo = sp.tile([128, F], dt.bfloat16, tag="vo", name="vo")
                nc.scalar.copy(vo[:], psv[:])
                nc.sync.dma_start(uvb[0][w * 128:(w + 1) * 128, :], uo[:])
                nc.sync.dma_start(uvb[1][w * 128:(w + 1) * 128, :], vo[:])
            for i in range(2):
                nc.gpsimd.collective_compute(
                    "AllGather", mybir.AluOpType.bypass, replica_groups=rg,
                    ins=[uvb[i][0:NC, :].opt()], outs=[uvf[i][0:N, :].opt()])

            # triplet select-mm streams: tout_s[e] = table[key_e] for edges
            # sorted by key window; host adds the u and v streams.
            def stream(table, nch_s, cinst, kq_d, out_d, tag):
                twc = {}
                sqc = {}

                def get_tw(w):
                    if w not in twc:
                        t = sp.tile([128, F], dt.bfloat16, tag=f"tw{tag}",
                                    name=f"tw{tag}")
                        base = min(w * 128, N - 128)
                        nc.scalar.dma_start(
                            t[:], table[base:base + 128, :])
                        twc.clear()
                        twc[w] = t
                    return twc[w]

                def get_sq(q):
                    if q not in sqc:
                        dlb = sp.tile([128, 512], dt.bfloat16,
                                      tag=f"db{tag}", name=f"db{tag}")
                        a = kq_d.ap()[q:q + 1, :]
                        bap = bass.AP(tensor=a.tensor, offset=a.offset,
                                      ap=[[0, 128], [1, 512]])
                        nc.scalar.dma_start(out=dlb[:], in_=bap)
                        sq = slabp.tile([128, 4, 128], dt.bfloat16,
                                        tag=f"sq{tag}", name=f"sq{tag}")
                        nc.vector.tensor_scalar(
                            sq[:].rearrange("p a b -> p (a b)"), dlb[:],
                            iotaf[:, 0:1], None,
                            op0=mybir.AluOpType.is_equal)
                        sqc.clear()
                        sqc[q] = sq
                    return sqc[q]

                for p0 in range(0, nch_s, PC):
                    blks = min(PC, nch_s - p0)
                    ot = msgp.tile([128, blks, F], dt.bfloat16,
                                   tag=f"o{tag}", name=f"o{tag}")
                    for b in range(blks):
                        lst = cinst[p0 + b]
                        pso = psoutp.tile([128, F], dt.float32,
                                          tag="po", name="po")
                        for (w, j, first, last) in lst:
                            sq = get_sq(j // 4)
                            tw = get_tw(w)
                            nc.tensor.matmul(pso[:], lhsT=sq[:, j % 4, :],
                                             rhs=tw[:],
                                             start=first, stop=last)
                        nc.scalar.copy(ot[:, b, :], pso[:])
                    nc.sync.dma_start(
                        out_d[p0 * 128:(p0 + blks) * 128, :]
                        .rearrange("(b p) o -> p b o", p=128),
                        ot[:])

            stream(uvf[0][:], nch_u, cinst_u, kqu_d, tu_d, "u")
            stream(uvf[1][:], nch_v, cinst_v, kqv_d, tv_d, "v")
    nc.compile()
    return nc


def kernel(**inputs):
    from concourse.bass_utils import run_bass_kernel_spmd

    x = np.asarray(inputs["x"], dtype=np.float32)
    ei = np.asarray(inputs["edge_index"], dtype=np.int64)
    et = np.asarray(inputs["edge_type"], dtype=np.int64)
    src, dst = ei[0], ei[1]
    cnt = np.bincount(dst * R + et, minlength=N * R)
    norm = (1.0 / np.maximum(cnt[dst * R + et], 1)).astype(np.float32)

    import time as _t
    _t0 = _t.time()
    nchunk, insts, idx_s, dlnm_s = _plan_agg(src, dst, et, norm)
    nch_u, cinst_u, ninst_u, plans_u = _plan_stream(src)
    nch_v, cinst_v, ninst_v, plans_v = _plan_stream(dst)
    print(f"[kernel] plan done {_t.time()-_t0:.1f}s", flush=True)
    nc = _build(nchunk, insts, (nch_u, cinst_u, ninst_u),
                (nch_v, cinst_v, ninst_v))
    print(f"[kernel] build+compile done {_t.time()-_t0:.1f}s", flush=True)

    x16 = x.astype(BF16)
    xpad = np.zeros((NPAD, F), dtype=BF16)
    w1 = np.asarray(inputs["W1"], np.float32).astype(BF16)
    w2 = np.asarray(inputs["W2"], np.float32).astype(BF16)
    r1 = np.asarray(inputs["root1"], np.float32).astype(BF16)
    r2 = np.asarray(inputs["root2"], np.float32).astype(BF16)
    wp = np.asarray(inputs["Wp"], np.float32)
    b1 = np.tile(np.asarray(inputs["b1"], np.float32).reshape(1, F), (128, 1))
    b2 = np.tile(np.asarray(inputs["b2"], np.float32).reshape(1, F), (128, 1))
    bp = np.tile(np.asarray(inputs["bp"], np.float32).reshape(1, F), (128, 1))

    in_maps = []
    for c in range(NCORES):
        xs = xpad.copy()
        xs[:NC] = x16[c * NC:(c + 1) * NC]
        in_maps.append({
            "x16": x16, "xsh": xs,
            "w1": w1, "w2": w2, "r1": r1, "r2": r2,
            "b1": b1, "b2": b2,
            "wpu": wp[:F].astype(BF16), "wpv": wp[F:].astype(BF16), "bp": bp,
            "idx_lo": idx_s[c][0], "idx_hi": idx_s[c][1],
            "dl": dlnm_s[c][0], "nm": dlnm_s[c][1],
            "kqu": plans_u[c][0], "kqv": plans_v[c][0],
        })
    import os
    res = None
    if os.environ.get("BASS_KERNEL_TRACE"):
        try:
            res = run_bass_kernel_spmd(nc, in_maps,
                                       core_ids=list(range(NCORES)), trace=True)
        except Exception:
            res = None
    if res is None:
        res = run_bass_kernel_spmd(nc, in_maps, core_ids=list(range(NCORES)))
    global LAST_EXEC_NS, LAST_TRACE
    LAST_EXEC_NS = res.exec_time_ns
    if res.instructions_and_trace is not None:
        LAST_TRACE = res.instructions_and_trace[1]
    out = np.zeros((E, F), dtype=np.float32)
    for c in range(NCORES):
        tu = np.asarray(res.results[c]["tu"]).astype(np.float32)
        ou = plans_u[c][1]
        valid = ou >= 0
        out[ou[valid]] = tu[valid]
        tv = np.asarray(res.results[c]["tv"]).astype(np.float32)
        ov = plans_v[c][1]
        valid = ov >= 0
        out[ov[valid]] += tv[valid]
    return out


# revision 37
# speedup vs baseline: 1.2561x; 1.2561x over previous
"""RGCN (2-layer, per-(dst,rel) mean aggregation) + triplet projection,
distributed over 8 Trainium2 NeuronCores (one SPMD Bass/Tile program).

Sharding: destination-node ranges (6250 nodes/core). Aggregate-first:
  y[dst,rel] = (1/cnt) * sum_{src} x[src]   built as one-hot "slab" matmuls
  accumulated in PSUM, then agg = sum_r y_r @ W_r + x @ root + b, ReLU,
  AllGather h. Triplet: u = h@Wp[:256]+bp, v = h@Wp[256:] per node,
  AllGather u & v, then out[e] = u[src_e] + v[dst_e] via dma_gather + add.

Gathers use gpsimd dma_gather (1024 rows per instruction, int16 indices).
The int16 limit (32767 < 50000 rows) is handled by splitting edges into lo
(src<32768, table base row 0) and hi (src>=32768, base row 17232) streams.

Edge slots are packed DENSELY: per-(w,r,reg) run sizes are the max edge
count over the 8 cores (no 128-rounding); runs share 128-row chunks. Each
(chunk, run) overlap is one matmul instance whose one-hot slab
(slab[e, dst%128] = norm, zero outside the run via a dl=200 sentinel) is
built on-chip with a single fused tensor_scalar (iota==dl)*norm op.
The instruction stream is identical on all cores; per-core variation lives
in the gather-index / dl / norm input tensors.
"""

import numpy as np
import ml_dtypes

BF16 = ml_dtypes.bfloat16

N, R, F, E, NCORES = 50000, 8, 256, 400000, 8
NC = N // NCORES             # 6250
W = (NC + 127) // 128        # 49 windows/core
NPAD = W * 128               # 6272
SPLIT = 32768
HIBASE = 17232               # hi idx = src - HIBASE (<= 32767)
PC = 8                       # gather piece = 8 chunks = 1024 rows
LAST_EXEC_NS = None
LAST_TRACE = None


def _wrap_idx(a):
    """[slots or n, 128] -> [128, n]: column j holds row j."""
    return np.ascontiguousarray(a.T)


def _wrap16(idx):
    """int16 idx [slots] -> [128, slots//16]: element i at [i%16, i//16],
    replicated across the 8 gpsimd core partition groups."""
    s = len(idx) // 16
    a = np.ascontiguousarray(idx.reshape(s, 16).T)
    return np.tile(a, (8, 1))


def _plan_agg(src, dst, et, norm):
    """Dense instance plan.  Returns:
    nchunk[2]      : chunk-grid length per region
    insts          : list of (w, r, reg, chunk, lo_in_chunk, hi_in_chunk,
                      first, last)  (common across cores)
    idx16 per core per region, dl/nm [128, n_inst] f32 per core
    """
    core = dst // NC
    percore = []
    counts = np.zeros((NCORES, W, R, 2), dtype=np.int64)
    for c in range(NCORES):
        m = np.where(core == c)[0]
        dl = dst[m] - c * NC
        w = dl >> 7
        reg = (src[m] >= SPLIT).astype(np.int64)
        percore.append((m, dl, w, reg))
        key = (w * R + et[m]) * 2 + reg
        counts[c] = np.bincount(key, minlength=W * R * 2).reshape(W, R, 2)
    S = counts.max(axis=0)                       # [W,R,2] common run sizes
    empty = S.sum(axis=2) == 0
    S[:, :, 0][empty] = 1                        # ensure >=1 instance per (w,r)

    # run offsets within each region's dense slot stream
    off = np.zeros((W, R, 2), dtype=np.int64)
    tot = [0, 0]
    for reg in range(2):
        acc = 0
        for w in range(W):
            for r in range(R):
                off[w, r, reg] = acc
                acc += S[w, r, reg]
        tot[reg] = acc
    nchunk = [(-(-tot[reg] // 128)) for reg in range(2)]

    # instance list: per (w, r): reg 0 then reg 1, chunks ascending
    insts = []
    for w in range(W):
        for r in range(R):
            items = []
            for reg in range(2):
                sz = int(S[w, r, reg])
                if sz == 0:
                    continue
                a, bnd = int(off[w, r, reg]), int(off[w, r, reg]) + sz
                for ch in range(a // 128, (bnd - 1) // 128 + 1):
                    lo = max(a, ch * 128) - ch * 128
                    hi = min(bnd, (ch + 1) * 128) - ch * 128
                    items.append([w, r, reg, ch, lo, hi, False, False])
            items[0][6] = True
            items[-1][7] = True
            insts.extend(items)
    n_inst = len(insts)

    # (w, r, reg, chunk) -> instance id, as a dense array
    nch = max(nchunk)
    lut = np.full((W * R * 2 * nch,), -1, dtype=np.int64)
    for j, (w_, r_, reg_, ch, lo, hi, _, _) in enumerate(insts):
        lut[((w_ * R + r_) * 2 + reg_) * nch + ch] = j

    idx_s, dlnm_s = [], []
    for c in range(NCORES):
        m, dl, w, regs = percore[c]
        r = et[m]
        ipair = []
        dlv = np.full((n_inst, 128), 200.0, dtype=np.float32)
        nmv = np.zeros((n_inst, 128), dtype=np.float32)
        for reg in range(2):
            slots = nchunk[reg] * 128
            idx = np.zeros(slots, dtype=np.int32)
            sel = regs == reg
            mm = m[sel]
            order = np.lexsort((dl[sel], r[sel] + R * w[sel]))
            mm = mm[order]
            wsel, rsel, dsel = w[sel][order], r[sel][order], dl[sel][order]
            runkey = wsel * R + rsel
            runstart = off[wsel, rsel, reg]
            o = np.arange(len(mm))
            starts = np.zeros(len(mm), dtype=np.int64)
            b = np.flatnonzero(np.diff(runkey)) + 1
            starts[b] = o[b]
            starts = np.maximum.accumulate(starts)
            pos = runstart + (o - starts)
            idx[pos] = src[mm] - reg * HIBASE
            ipair.append(_wrap16(idx.astype(np.int16)))
            j = lut[((wsel * R + rsel) * 2 + reg) * nch + (pos >> 7)]
            assert (j >= 0).all()
            dlv[j, pos & 127] = dsel & 127
            nmv[j, pos & 127] = norm[mm]
        idx_s.append(ipair)
        dlnm_s.append((_wrap_idx(dlv.astype(BF16)), _wrap_idx(nmv.astype(BF16))))
    return nchunk, insts, idx_s, dlnm_s


WG = (N + 127) // 128        # 391 global node windows


def _plan_stream(key):
    """Plan one triplet select-mm stream over the per-core edge shards,
    sorted by the global node-window of `key` (src or dst).

    Returns nch (chunk count), chunk_insts (per chunk: list of
    (w, first, last)), n_inst, and per-core (kq, orig):
      kq   [n_quad, 512] bf16 : instance keys (key % 128, sentinel 200),
                                4 instances per quad row
      orig [nch*128] int64    : slot -> original edge id (-1 = pad)
    """
    EC = E // NCORES
    counts = np.zeros((NCORES, WG), dtype=np.int64)
    percore = []
    for c in range(NCORES):
        ids = np.arange(c * EC, (c + 1) * EC)
        kw = key[ids] >> 7
        order = np.argsort(kw, kind="stable")
        ids = ids[order]
        percore.append(ids)
        counts[c] = np.bincount(kw[order], minlength=WG)
    S = counts.max(axis=0)
    off = np.concatenate([[0], np.cumsum(S)]).astype(np.int64)
    tot = int(off[-1])
    nch = -(-tot // 128)

    insts = []          # (w, ch, first_of_chunk, last_of_chunk)
    chunk_insts = [[] for _ in range(nch)]
    for w in range(WG):
        if S[w] == 0:
            continue
        a, bnd = int(off[w]), int(off[w + 1])
        for ch in range(a // 128, (bnd - 1) // 128 + 1):
            chunk_insts[ch].append(w)
    n_inst = 0
    inst_of = {}
    out = []
    for ch in range(nch):
        ws = chunk_insts[ch]
        if not ws:
            out.append([])
            continue
        lst = []
        for k, w in enumerate(ws):
            inst_of[(w, ch)] = n_inst
            lst.append((w, n_inst, k == 0, k == len(ws) - 1))
            n_inst += 1
        out.append(lst)
    n_quad = -(-n_inst // 4)

    plans = []
    for c in range(NCORES):
        ids = percore[c]
        kw = key[ids] >> 7
        # dense position within the common offsets
        o = np.arange(len(ids))
        starts = np.zeros(len(ids), dtype=np.int64)
        b = np.flatnonzero(np.diff(kw)) + 1
        starts[b] = o[b]
        starts = np.maximum.accumulate(starts)
        pos = off[kw] + (o - starts)
        orig = np.full(nch * 128, -1, dtype=np.int64)
        orig[pos] = ids
        kv = np.full((n_inst, 128), 200.0, dtype=np.float32)
        j = np.array([inst_of[(int(w_), int(p_) >> 7)]
                      for w_, p_ in zip(kw, pos)], dtype=np.int64)
        # key offset within its window tile; the last window's tile is
        # loaded from rows [N-128, N) so shift its keys accordingly
        kv[j, pos & 127] = key[ids] - np.minimum(kw * 128, N - 128)
        n_oct = -(-n_inst // 8)
        kq = np.full((n_oct * 8, 128), 200.0, dtype=np.float32)
        kq[:n_inst] = kv
        kq = kq.reshape(n_oct, 1024).astype(BF16)
        plans.append((kq, orig))
    return nch, out, n_inst, plans


def _build(nchunk, insts, su, sv):
    import concourse.bass as bass
    import concourse.bacc as bacc
    import concourse.mybir as mybir
    import concourse.tile as tile

    dt = mybir.dt
    nc = bacc.Bacc("TRN2", target_bir_lowering=False, debug=False,
                   num_devices=NCORES)
    AF = mybir.ActivationFunctionType
    n_inst = len(insts)
    nch_u, cinst_u, ninst_u = su
    nch_v, cinst_v, ninst_v = sv
    mq_u = (((ninst_u + 3) // 4) + 127) // 128
    mq_v = (((ninst_v + 3) // 4) + 127) // 128
    NPADG = WG * 128

    x16 = nc.dram_tensor("x16", [N, F], dt.bfloat16, kind="ExternalInput")
    xsh = nc.dram_tensor("xsh", [NPAD, F], dt.bfloat16, kind="ExternalInput")
    w1d = nc.dram_tensor("w1", [R, F, F], dt.bfloat16, kind="ExternalInput")
    w2d = nc.dram_tensor("w2", [R, F, F], dt.bfloat16, kind="ExternalInput")
    r1d = nc.dram_tensor("r1", [F, F], dt.bfloat16, kind="ExternalInput")
    r2d = nc.dram_tensor("r2", [F, F], dt.bfloat16, kind="ExternalInput")
    b1d = nc.dram_tensor("b1", [128, F], dt.float32, kind="ExternalInput")
    b2d = nc.dram_tensor("b2", [128, F], dt.float32, kind="ExternalInput")
    wpud = nc.dram_tensor("wpu", [F, F], dt.bfloat16, kind="ExternalInput")
    wpvd = nc.dram_tensor("wpv", [F, F], dt.bfloat16, kind="ExternalInput")
    bpd = nc.dram_tensor("bp", [128, F], dt.float32, kind="ExternalInput")
    ilo_d = nc.dram_tensor("idx_lo", [128, nchunk[0] * 8], dt.int16, kind="ExternalInput")
    ihi_d = nc.dram_tensor("idx_hi", [128, nchunk[1] * 8], dt.int16, kind="ExternalInput")
    dl_d = nc.dram_tensor("dl", [128, n_inst], dt.bfloat16, kind="ExternalInput")
    nm_d = nc.dram_tensor("nm", [128, n_inst], dt.bfloat16, kind="ExternalInput")
    kqu_d = nc.dram_tensor("kqu", [(ninst_u + 7) // 8, 1024], dt.bfloat16, kind="ExternalInput")
    kqv_d = nc.dram_tensor("kqv", [(ninst_v + 7) // 8, 1024], dt.bfloat16, kind="ExternalInput")
    tu_d = nc.dram_tensor("tu", [nch_u * 128, F], dt.bfloat16, kind="ExternalOutput")
    tv_d = nc.dram_tensor("tv", [nch_v * 128, F], dt.bfloat16, kind="ExternalOutput")

    rg = [list(range(NCORES))]

    with tile.TileContext(nc) as tc:
        with (
            tc.tile_pool(name="const", bufs=1) as cp,
            tc.tile_pool(name="msg", bufs=3) as msgp,
            tc.tile_pool(name="slab", bufs=4) as slabp,
            tc.tile_pool(name="yw", bufs=2) as yp,
            tc.tile_pool(name="small", bufs=4) as sp,
            tc.tile_pool(name="ps", bufs=1, space="PSUM") as psp,
            tc.tile_pool(name="psagg", bufs=1, space="PSUM") as psaggp,
            tc.tile_pool(name="psout", bufs=2, space="PSUM") as psoutp,
            tc.tile_pool(name="dram", bufs=1, space="DRAM") as dram,
        ):
            w_sb = [cp.tile([128, 16, F], dt.bfloat16, tag=f"w{i}", name=f"w{i}") for i in range(2)]
            nc.sync.dma_start(w_sb[0][:], w1d.ap().rearrange("r (h p) o -> p (r h) o", p=128))
            nc.sync.dma_start(w_sb[1][:], w2d.ap().rearrange("r (h p) o -> p (r h) o", p=128))
            rt_sb = [cp.tile([128, 2, F], dt.bfloat16, tag=f"rt{i}", name=f"rt{i}") for i in range(2)]
            nc.sync.dma_start(rt_sb[0][:], r1d.ap().rearrange("(h p) o -> p h o", p=128))
            nc.sync.dma_start(rt_sb[1][:], r2d.ap().rearrange("(h p) o -> p h o", p=128))
            b_sb = [cp.tile([128, F], dt.float32, tag=f"b{i}", name=f"b{i}") for i in range(2)]
            nc.sync.dma_start(b_sb[0][:], b1d[:])
            nc.sync.dma_start(b_sb[1][:], b2d[:])
            wpu_sb = cp.tile([128, 2, F], dt.bfloat16, tag="wpu", name="wpu")
            wpv_sb = cp.tile([128, 2, F], dt.bfloat16, tag="wpv", name="wpv")
            nc.sync.dma_start(wpu_sb[:], wpud.ap().rearrange("(h p) o -> p h o", p=128))
            nc.sync.dma_start(wpv_sb[:], wpvd.ap().rearrange("(h p) o -> p h o", p=128))
            bp_sb = cp.tile([128, F], dt.float32, tag="bp", name="bp")
            nc.sync.dma_start(bp_sb[:], bpd[:])
            ilo_sb = cp.tile([128, nchunk[0] * 8], dt.int16, tag="ilo", name="ilo")
            ihi_sb = cp.tile([128, nchunk[1] * 8], dt.int16, tag="ihi", name="ihi")
            nc.sync.dma_start(ilo_sb[:], ilo_d[:])
            nc.sync.dma_start(ihi_sb[:], ihi_d[:])
            dl_sb = cp.tile([128, n_inst], dt.bfloat16, tag="dl", name="dl")
            nm_sb = cp.tile([128, n_inst], dt.bfloat16, tag="nm", name="nm")
            nc.sync.dma_start(dl_sb[:], dl_d[:])
            nc.sync.dma_start(nm_sb[:], nm_d[:])
            iota_sb = cp.tile([128, 128], dt.bfloat16, tag="iota", name="iota")
            nc.gpsimd.iota(iota_sb[:], pattern=[[1, 128]], channel_multiplier=0,
                           allow_small_or_imprecise_dtypes=True)
            iotai = cp.tile([128, 1], dt.int32, tag="ioi", name="ioi")
            nc.gpsimd.iota(iotai[:], pattern=[[0, 1]], channel_multiplier=1)
            iotaf = cp.tile([128, 1], dt.float32, tag="iof", name="iof")
            nc.vector.tensor_copy(iotaf[:], iotai[:])
            zt_sb = cp.tile([128, F], dt.bfloat16, tag="zt", name="zt")
            nc.vector.memset(zt_sb[:], 0.0)

            h1b = dram.tile([NPAD, F], dt.bfloat16, tag="h1b", name="h1b")
            h2b = dram.tile([NPAD, F], dt.bfloat16, tag="h2b", name="h2b")
            h1f = dram.tile([N, F], dt.bfloat16, addr_space="Shared", tag="h1f", name="h1f")
            uvb = [dram.tile([NPAD, F], dt.bfloat16, tag=f"uvb{i}", name=f"uvb{i}") for i in range(2)]
            uvf = [dram.tile([N, F], dt.bfloat16, addr_space="Shared", tag=f"uvf{i}", name=f"uvf{i}")
                   for i in range(2)]

            def gather_piece(table, idx_sb, c0, nblk, tag):
                """Gather chunks [c0, c0+nblk) of a stream into an SBUF tile
                with one dma_gather (1024 rows max)."""
                t = msgp.tile([128, nblk, F], dt.bfloat16, tag=tag, name=tag)
                ni = nblk * 128
                nc.gpsimd.dma_gather(
                    t[:], table, idx_sb[:, c0 * 8:(c0 + nblk) * 8],
                    ni, ni, F)
                return t

            GS = 8   # slab instances built per DVE op pair

            def layer(li, tables, rootsrc, hout):
                idx_sb = (ilo_sb, ihi_sb)
                pieces = [{}, {}]   # region -> piece idx -> msg_tile
                sgroups = {}        # group idx -> slab tile [128, GS, 128]

                def get_piece(reg, p):
                    if p not in pieces[reg]:
                        nblk = min(PC, nchunk[reg] - p * PC)
                        mt = gather_piece(tables[reg], idx_sb[reg], p * PC,
                                          nblk, f"m{reg}")
                        pieces[reg] = {p: mt}  # keep only latest
                    return pieces[reg][p]

                def get_slab(j):
                    g = j // GS
                    if g not in sgroups:
                        ng = min(GS, len(insts) - g * GS)
                        st = slabp.tile([128, ng, 128], dt.bfloat16,
                                        tag="st", name="st")
                        nc.vector.tensor_tensor(
                            st[:],
                            iota_sb[:, None, :].broadcast_to([128, ng, 128]),
                            dl_sb[:, g * GS:g * GS + ng, None]
                            .broadcast_to([128, ng, 128]),
                            op=mybir.AluOpType.is_equal)
                        nc.vector.tensor_tensor(
                            st[:], st[:],
                            nm_sb[:, g * GS:g * GS + ng, None]
                            .broadcast_to([128, ng, 128]),
                            op=mybir.AluOpType.mult)
                        sgroups.clear()
                        sgroups[g] = st
                    return sgroups[g]

                ii = 0
                for w in range(W):
                    ps = [[psp.tile([128, 512], dt.float32, tag=f"ps{fh}{q}", name=f"ps{fh}{q}")
                           for q in range(2)] for fh in range(2)]
                    while ii < len(insts) and insts[ii][0] == w:
                        _, r, reg, ch, lo, hi, first, last = insts[ii]
                        p, b = divmod(ch, PC)
                        mt = get_piece(reg, p)
                        st = get_slab(ii)
                        for fh in range(2):
                            nc.tensor.matmul(
                                ps[fh][r // 4][:, (r % 4) * 128:(r % 4) * 128 + 128],
                                lhsT=mt[:, b, fh * 128:(fh + 1) * 128],
                                rhs=st[:, ii - (ii // GS) * GS, :],
                                start=first, stop=last)
                        ii += 1
                    yw = yp.tile([128, 2048], dt.bfloat16, tag="yw", name="yw")
                    for fh in range(2):
                        for q in range(2):
                            eng = nc.vector if q == 0 else nc.scalar
                            (eng.tensor_copy if q == 0 else eng.copy)(
                                yw[:, (fh * 2 + q) * 512:(fh * 2 + q + 1) * 512],
                                ps[fh][q][:])
                    xt = sp.tile([128, 2, 128], dt.bfloat16, tag="xt", name="xt")
                    for fh in range(2):
                        nc.sync.dma_start(
                            xt[:, fh, :],
                            rootsrc[w * 128:(w + 1) * 128, fh * 128:(fh + 1) * 128],
                            transpose=True)
                    agg = psaggp.tile([128, F], dt.float32, tag="agg", name="agg")
                    for r in range(R):
                        for fh in range(2):
                            nc.tensor.matmul(
                                agg[:], lhsT=yw[:, (fh * 8 + r) * 128:(fh * 8 + r + 1) * 128],
                                rhs=w_sb[li][:, r * 2 + fh, :],
                                start=(r == 0 and fh == 0), stop=False)
                    for fh in range(2):
                        nc.tensor.matmul(agg[:], lhsT=xt[:, fh, :],
                                         rhs=rt_sb[li][:, fh, :],
                                         start=False, stop=(fh == 1))
                    hf = sp.tile([128, F], dt.float32, tag="hf", name="hf")
                    nc.vector.tensor_tensor(hf[:], agg[:],
                                            b_sb[li][:],
                                            op=mybir.AluOpType.add)
                    hw = sp.tile([128, F], dt.bfloat16, tag="hw", name="hw")
                    nc.scalar.activation(hw[:], hf[:], AF.Relu)
                    nc.sync.dma_start(hout[w * 128:(w + 1) * 128, :], hw[:])

            layer(0, (x16.ap(), x16.ap()[HIBASE:, :]), xsh.ap(), h1b)
            nc.gpsimd.collective_compute(
                "AllGather", mybir.AluOpType.bypass, replica_groups=rg,
                ins=[h1b[0:NC, :].opt()], outs=[h1f[:].opt()])
            layer(1, (h1f[:], h1f[HIBASE:, :]), h1b[:], h2b)

            # triplet projections u, v per node tile
            for w in range(W):
                ht = sp.tile([128, 2, 128], dt.bfloat16, tag="ht", name="ht")
                for fh in range(2):
                    nc.sync.dma_start(
                        ht[:, fh, :],
                        h2b[w * 128:(w + 1) * 128, fh * 128:(fh + 1) * 128],
                        transpose=True)
                psu = psaggp.tile([128, F], dt.float32, tag="agg", name="psu")
                psv = psaggp.tile([128, F], dt.float32, tag="psv", name="psv")
                for fh in range(2):
                    nc.tensor.matmul(psu[:], lhsT=ht[:, fh, :], rhs=wpu_sb[:, fh, :],
                                     start=(fh == 0), stop=(fh == 1))
                    nc.tensor.matmul(psv[:], lhsT=ht[:, fh, :], rhs=wpv_sb[:, fh, :],
                                     start=(fh == 0), stop=(fh == 1))
                uo = sp.tile([128, F], dt.bfloat16, tag="uo", name="uo")
                nc.vector.tensor_tensor(uo[:], psu[:],
                                        bp_sb[:],
                                        op=mybir.AluOpType.add)
                vo = sp.tile([128, F], dt.bfloat16, tag="vo", name="vo")
                nc.scalar.copy(vo[:], psv[:])
                nc.sync.dma_start(uvb[0][w * 128:(w + 1) * 128, :], uo[:])
                nc.sync.dma_start(uvb[1][w * 128:(w + 1) * 128, :], vo[:])
            for i in range(2):
                nc.gpsimd.collective_compute(
                    "AllGather", mybir.AluOpType.bypass, replica_groups=rg,
                    ins=[uvb[i][0:NC, :].opt()], outs=[uvf[i][0:N, :].opt()])

            # triplet select-mm streams: tout_s[e] = table[key_e] for edges
            # sorted by key window; host adds the u and v streams.
            WQA = (WG // 4) * 4   # windows below this load as aligned quads

            def stream(table, nch_s, cinst, kq_d, out_d, tag):
                twc = {}
                sqc = {}

                def get_tw(w):
                    key = w // 4 if w < WQA else ('s', w)
                    if key not in twc:
                        if w < WQA:
                            t = sp.tile([128, 4, F], dt.bfloat16,
                                        tag=f"tw{tag}", name=f"tw{tag}")
                            q = w // 4
                            nc.sync.dma_start(
                                t[:], table[q * 512:(q + 1) * 512, :]
                                .rearrange("(b p) o -> p b o", p=128))
                        else:
                            t = sp.tile([128, 1, F], dt.bfloat16,
                                        tag=f"tws{tag}", name=f"tws{tag}")
                            base = min(w * 128, N - 128)
                            nc.sync.dma_start(
                                t[:, 0, :], table[base:base + 128, :])
                        twc.clear()
                        twc[key] = t
                    t = twc[key]
                    return t[:, w % 4 if w < WQA else 0, :]

                def get_sq(o):
                    if o not in sqc:
                        dlb = sp.tile([128, 1024], dt.bfloat16,
                                      tag=f"db{tag}", name=f"db{tag}")
                        a = kq_d.ap()[o:o + 1, :]
                        bap = bass.AP(tensor=a.tensor, offset=a.offset,
                                      ap=[[0, 128], [1, 1024]])
                        nc.scalar.dma_start(out=dlb[:], in_=bap)
                        sq = slabp.tile([128, 8, 128], dt.bfloat16,
                                        tag=f"sq{tag}", name=f"sq{tag}")
                        nc.vector.tensor_scalar(
                            sq[:].rearrange("p a b -> p (a b)"), dlb[:],
                            iotaf[:, 0:1], None,
                            op0=mybir.AluOpType.is_equal)
                        sqc.clear()
                        sqc[o] = sq
                    return sqc[o]

                for p0 in range(0, nch_s, PC):
                    blks = min(PC, nch_s - p0)
                    ot = msgp.tile([128, blks, F], dt.bfloat16,
                                   tag=f"o{tag}", name=f"o{tag}")
                    for b in range(blks):
                        lst = cinst[p0 + b]
                        pso = psoutp.tile([128, F], dt.float32,
                                          tag="po", name="po")
                        for (w, j, first, last) in lst:
                            sq = get_sq(j // 8)
                            tw = get_tw(w)
                            nc.tensor.matmul(pso[:], lhsT=sq[:, j % 8, :],
                                             rhs=tw,
                                             start=first, stop=last)
                        eng = nc.scalar if b % 2 else nc.vector
                        (eng.copy if b % 2 else eng.tensor_copy)(
                            ot[:, b, :], pso[:])
                    nc.sync.dma_start(
                        out_d[p0 * 128:(p0 + blks) * 128, :]
                        .rearrange("(b p) o -> p b o", p=128),
                        ot[:])

            stream(uvf[0][:], nch_u, cinst_u, kqu_d, tu_d, "u")
            stream(uvf[1][:], nch_v, cinst_v, kqv_d, tv_d, "v")
    nc.compile()
    return nc


def kernel(**inputs):
    from concourse.bass_utils import run_bass_kernel_spmd

    x = np.asarray(inputs["x"], dtype=np.float32)
    ei = np.asarray(inputs["edge_index"], dtype=np.int64)
    et = np.asarray(inputs["edge_type"], dtype=np.int64)
    src, dst = ei[0], ei[1]
    cnt = np.bincount(dst * R + et, minlength=N * R)
    norm = (1.0 / np.maximum(cnt[dst * R + et], 1)).astype(np.float32)

    import time as _t
    _t0 = _t.time()
    nchunk, insts, idx_s, dlnm_s = _plan_agg(src, dst, et, norm)
    nch_u, cinst_u, ninst_u, plans_u = _plan_stream(src)
    nch_v, cinst_v, ninst_v, plans_v = _plan_stream(dst)
    print(f"[kernel] plan done {_t.time()-_t0:.1f}s", flush=True)
    nc = _build(nchunk, insts, (nch_u, cinst_u, ninst_u),
                (nch_v, cinst_v, ninst_v))
    print(f"[kernel] build+compile done {_t.time()-_t0:.1f}s", flush=True)

    x16 = x.astype(BF16)
    xpad = np.zeros((NPAD, F), dtype=BF16)
    w1 = np.asarray(inputs["W1"], np.float32).astype(BF16)
    w2 = np.asarray(inputs["W2"], np.float32).astype(BF16)
    r1 = np.asarray(inputs["root1"], np.float32).astype(BF16)
    r2 = np.asarray(inputs["root2"], np.float32).astype(BF16)
    wp = np.asarray(inputs["Wp"], np.float32)
    b1 = np.tile(np.asarray(inputs["b1"], np.float32).reshape(1, F), (128, 1))
    b2 = np.tile(np.asarray(inputs["b2"], np.float32).reshape(1, F), (128, 1))
    bp = np.tile(np.asarray(inputs["bp"], np.float32).reshape(1, F), (128, 1))

    in_maps = []
    for c in range(NCORES):
        xs = xpad.copy()
        xs[:NC] = x16[c * NC:(c + 1) * NC]
        in_maps.append({
            "x16": x16, "xsh": xs,
            "w1": w1, "w2": w2, "r1": r1, "r2": r2,
            "b1": b1, "b2": b2,
            "wpu": wp[:F].astype(BF16), "wpv": wp[F:].astype(BF16), "bp": bp,
            "idx_lo": idx_s[c][0], "idx_hi": idx_s[c][1],
            "dl": dlnm_s[c][0], "nm": dlnm_s[c][1],
            "kqu": plans_u[c][0], "kqv": plans_v[c][0],
        })
    import os
    res = None
    if os.environ.get("BASS_KERNEL_TRACE"):
        try:
            res = run_bass_kernel_spmd(nc, in_maps,
                                       core_ids=list(range(NCORES)), trace=True)
        except Exception:
            res = None
    if res is None:
        res = run_bass_kernel_spmd(nc, in_maps, core_ids=list(range(NCORES)))
    global LAST_EXEC_NS, LAST_TRACE
    LAST_EXEC_NS = res.exec_time_ns
    if res.instructions_and_trace is not None:
        LAST_TRACE = res.instructions_and_trace[1]
    out = np.zeros((E, F), dtype=np.float32)
    for c in range(NCORES):
        tu = np.asarray(res.results[c]["tu"]).astype(np.float32)
        ou = plans_u[c][1]
        valid = ou >= 0
        out[ou[valid]] = tu[valid]
        tv = np.asarray(res.results[c]["tv"]).astype(np.float32)
        ov = plans_v[c][1]
        valid = ov >= 0
        out[ov[valid]] += tv[valid]
    return out
